# revision 49
# baseline (speedup 1.0000x reference)
"""BitMGQA forward on 8 trn2 NeuronCores — collective-free.

Core c owns batch b=c//4 and query rows (c%4)*512:(c%4+1)*512. Each core
recomputes the K/V projections for its batch's FULL 2048-key sequence
locally (instead of all-gathering K/V across the batch group), so there are
no cross-core dependencies at all: each core's on-device instruction span
is just its own work, immune to multi-core launch skew.

All matmul operands are bf16 (fp32 PSUM accumulation). LayerNorm is folded
into the output projection: with z = y @ (ln_w*WoT), s_w = ln_w @ WoT,
lnbo = ln_b @ WoT + bo, the output is rstd*(z - mu*s_w) + lnbo, so the
-mu*s_w term rides the z PSUM accumulation and the per-row rstd scale plus
lnbo add are two DVE ops per output tile. Outputs are disjoint row
slices -> host concat.
"""

import numpy as np

import concourse.bacc as bacc
import concourse.mybir as mybir
import concourse.tile as tile
from concourse.bass_utils import run_bass_kernel_spmd

B, T, C = 2, 2048, 2048
H, KV = 16, 4
HD = C // H  # 128
KVC = HD * KV  # 512
EPS = 1e-5
R = 512  # query rows per core
N_CORES = 8
SCALE = 1.0 / np.sqrt(HD)

F32 = mybir.dt.float32
F32R = mybir.dt.float32r
BF16 = mybir.dt.bfloat16
AF = mybir.ActivationFunctionType
ALU = mybir.AluOpType


def build_kernel(loop_n=1):
    nc = bacc.Bacc(
        "TRN2", target_bir_lowering=False, debug=False, num_devices=N_CORES
    )

    # Per-core inputs (host pre-transposed/tiled, see kernel() below)
    xq_d = nc.dram_tensor("xq", [128, 16, R], BF16, kind="ExternalInput").ap()
    xk_d = nc.dram_tensor("xk", [128, 16, T], BF16, kind="ExternalInput").ap()
    xv_d = nc.dram_tensor("xv", [128, 16, T], BF16, kind="ExternalInput").ap()
    # wq[j] = [128, 16, 128] (c-within-tile, c-tile, ch) for ch-tile j
    wq_d = nc.dram_tensor("wq", [16, 128, 16, 128], BF16, kind="ExternalInput").ap()
    wk_d = nc.dram_tensor("wk", [128, 4, 16, 128], BF16, kind="ExternalInput").ap()
    wv_d = nc.dram_tensor("wv", [128, 16, KVC], BF16, kind="ExternalInput").ap()
    wo_d = nc.dram_tensor("wo", [4, 128, 16, 512], BF16, kind="ExternalInput").ap()
    # packed consts: cf = [bq(16) | bk(4)] f32; crb = [bv(512) | sw(2048) |
    # lnbo(2048) | ones(512)] bf16
    cf_d = nc.dram_tensor("cf", [128, 20], F32, kind="ExternalInput").ap()
    crb_d = nc.dram_tensor("crb", [1, 5120], BF16, kind="ExternalInput").ap()
    onesb_d = nc.dram_tensor("onesb", [128, 1], BF16, kind="ExternalInput").ap()
    onesr_d = nc.dram_tensor("onesr", [1, 512], F32R, kind="ExternalInput").ap()

    out_d = nc.dram_tensor("out", [R, C], F32, kind="ExternalOutput").ap()

    from contextlib import ExitStack

    with tile.TileContext(nc) as tc:
        with ExitStack() as stack:
            ep = stack.enter_context
            consts = ep(tc.tile_pool(name="consts", bufs=1))
            dram = ep(tc.tile_pool(name="dram", bufs=1, space="DRAM"))
            xt = ep(tc.tile_pool(name="xt", bufs=2))            # [128,16,512] bf16
            wkb_pool = ep(tc.tile_pool(name="wkb", bufs=1))     # [128,4,16,128] bf16
            wqb_pool = ep(tc.tile_pool(name="wqb", bufs=2))     # [128,16,128] bf16
            wvb_pool = ep(tc.tile_pool(name="wvb", bufs=1))     # [128,16,512] bf16
            ktf_pool = ep(tc.tile_pool(name="ktf", bufs=4))     # [128,2048] bf16
            vfb_pool = ep(tc.tile_pool(name="vfb", bufs=16))    # [128,512] bf16
            qtb_pool = ep(tc.tile_pool(name="qtb", bufs=16))    # [128,512] bf16
            blk = ep(tc.tile_pool(name="blk", bufs=12))         # [128,1024] bf16
            blkf = ep(tc.tile_pool(name="blkf", bufs=3))        # [128,512] f32
            lnbo_pool = ep(tc.tile_pool(name="lnbo", bufs=4))   # [128,512] bf16
            ytp = ep(tc.tile_pool(name="ytp", bufs=1))          # [128,16,512] bf16
            s1 = ep(tc.tile_pool(name="s1", bufs=3))            # [1,512] f32
            rsc = ep(tc.tile_pool(name="rsc", bufs=1))          # [128,4] f32
            # PSUM: ps slots are [128,1024] (2 banks); 2x2 + 2 + 2 = 8 banks
            ps = ep(tc.tile_pool(name="ps", bufs=2, space="PSUM"))    # [128,1024]
            psy = ep(tc.tile_pool(name="psy", bufs=2, space="PSUM"))  # [128,512]
            pss = ep(tc.tile_pool(name="pss", bufs=2, space="PSUM"))  # [1,512]
            for _it in range(loop_n):
                # ---- bulk input streams first; wk on the ACT ring so it
                # transfers in parallel with xkq0 on the SP ring. Both are
                # split so the first K-proj group's operands land first.
                wk_sb = wkb_pool.tile([128, 4, 16, 128], BF16, tag="wkb")
                nc.scalar.dma_start(out=wk_sb[:, 0:1], in_=wk_d[:, 0:1])
                nc.scalar.dma_start(out=wk_sb[:, 1:4], in_=wk_d[:, 1:4])
                wkb = [wk_sb[:, g] for g in range(4)]
                xkq0 = xt.tile([128, 16, 512], BF16, tag="xt", name="xkq0")
                nc.sync.dma_start(out=xkq0[:, 0:8], in_=xk_d[:, 0:8, 0:512])
                nc.sync.dma_start(out=xkq0[:, 8:16], in_=xk_d[:, 8:16, 0:512])

                # ---- consts on the ACT HWDGE ring (parallel to SP's) ----
                cf = consts.tile([128, 20], F32)
                nc.scalar.dma_start(out=cf[:], in_=cf_d[:])
                bq_sb = cf[:, 0:16]
                bk_sb = cf[:, 16:20]
                crb = consts.tile([1, 5120], BF16)
                nc.scalar.dma_start(out=crb[:], in_=crb_d[:])
                bv_sb = crb[:, 0:KVC]
                sw_sb = crb[:, KVC:KVC + C]
                lnbo_sb = crb[:, KVC + C:KVC + 2 * C]
                ones_rowb = crb[:, KVC + 2 * C:KVC + 2 * C + 512]
                ones_colb = consts.tile([128, 1], BF16)
                nc.scalar.dma_start(out=ones_colb[:], in_=onesb_d[:])
                ones_row = consts.tile([1, 512], F32R)
                nc.scalar.dma_start(out=ones_row[:], in_=onesr_d[:])

                # ---- K projection (k^T layout), full T, streamed t-quarters ----
                ktfg = []
                for g in range(4):
                    ktfg.append(ktf_pool.tile([128, T], BF16, tag="ktfg",
                                              name=f"ktfg{g}"))
                for tq in range(4):
                    if tq == 0:
                        xkq = xkq0
                    else:
                        xkq = xt.tile([128, 16, 512], BF16, tag="xt",
                                      name=f"xkq{tq}")
                        nc.sync.dma_start(
                            out=xkq[:], in_=xk_d[:, :, tq * 512:(tq + 1) * 512]
                        )
                    for g in range(4):
                        ps_k = ps.tile([128, 512], F32, tag="ps")
                        for i in range(16):
                            nc.tensor.matmul(
                                ps_k[:], wkb[g][:, i, :], xkq[:, i, :],
                                start=(i == 0), stop=(i == 15),
                                skip_group_check=True,
                            )
                        nc.scalar.activation(
                            ktfg[g][:, tq * 512:(tq + 1) * 512], ps_k[:],
                            AF.Identity, bias=bk_sb[:, g:g + 1],
                        )

                # ---- V projection (natural layout), full T, streamed ----
                wv_sb = wvb_pool.tile([128, 16, KVC], BF16)
                nc.sync.dma_start(out=wv_sb[:], in_=wv_d[:])
                vfr = []
                for tq in range(4):
                    xvq = xt.tile([128, 16, 512], BF16, tag="xt", name=f"xvq{tq}")
                    nc.sync.dma_start(out=xvq[:], in_=xv_d[:, :, tq * 512:(tq + 1) * 512])
                    for tt in range(4):
                        ps_v = ps.tile([128, 512], F32, tag="ps")
                        nc.tensor.matmul(
                            ps_v[:], ones_rowb[0:1, 0:128], bv_sb[0:1, :],
                            start=True, stop=False,
                        )
                        for i in range(16):
                            nc.tensor.matmul(
                                ps_v[:], xvq[:, i, tt * 128:(tt + 1) * 128],
                                wv_sb[:, i, :], start=False, stop=(i == 15),
                                skip_group_check=True,
                            )
                        vt = vfb_pool.tile([128, KVC], BF16, tag="vfr",
                                           name=f"vfr{tq*4+tt}")
                        nc.scalar.activation(vt[:], ps_v[:], AF.Copy)
                        vfr.append(vt)

                # ---- Q projection is interleaved into the attention loop ----
                xq = xt.tile([128, 16, R], BF16, tag="xt", name="xq")
                nc.sync.dma_start(out=xq[:], in_=xq_d[:])

                def qproj(j):
                    wqb = wqb_pool.tile([128, 16, 128], BF16, tag="wqb")
                    nc.sync.dma_start(out=wqb[:], in_=wq_d[j])
                    ps_q = ps.tile([128, 512], F32, tag="ps")
                    for i in range(16):
                        nc.tensor.matmul(
                            ps_q[:], wqb[:, i, :], xq[:, i, :],
                            start=(i == 0), stop=(i == 15), skip_group_check=True,
                        )
                    qh = qtb_pool.tile([128, R], BF16, tag="qtbh", name=f"qtbh{j}")
                    nc.scalar.activation(
                        qh[:], ps_q[:], AF.Identity, bias=bq_sb[:, j:j + 1]
                    )
                    return qh

                # first two heads' q up front so attention can start
                qtbh = [qproj(0), qproj(1)]

                # ---- prefetch first Wo blocks; lnbo broadcast tiles ----
                wob = [None] * 4
                for jb in range(2):
                    wob[jb] = xt.tile([128, 16, 512], BF16, tag="xt",
                                      name=f"wob{jb}")
                    nc.sync.dma_start(out=wob[jb][:], in_=wo_d[jb])
                lnbo_b = []
                for jb in range(4):
                    ps_l = ps.tile([128, 512], F32, tag="ps")
                    nc.tensor.matmul(
                        ps_l[:], ones_rowb[0:1, 0:128],
                        lnbo_sb[0:1, jb * 512:(jb + 1) * 512],
                        start=True, stop=True,
                    )
                    lb = lnbo_pool.tile([128, 512], BF16, tag="lnbo",
                                        name=f"lnbo{jb}")
                    nc.scalar.activation(lb[:], ps_l[:], AF.Copy)
                    lnbo_b.append(lb)

                # ---- attention: paired score tiles, DVE softmax-sum tree.
                # Software-pipelined: head h's tail (tree -> S -> 1/S ->
                # bcast -> normalize) is emitted AFTER head h+1's score/AV
                # block, so the PE queue always holds independent matmuls
                # while the DVE tree catches up (no head-of-line stalls).
                yt = ytp.tile([128, 16, R], BF16)

                def attn_head(h):
                    g = h // 4
                    gs0, gs1 = g * 128, (g + 1) * 128
                    ps_y = psy.tile([128, 512], F32, tag="psy")
                    pairs = []
                    for p in range(8):
                        ps_s = ps.tile([128, 1024], F32, tag="ps")
                        nc.tensor.matmul(
                            ps_s[:, 0:512],
                            ktfg[g][:, (2 * p) * 128:(2 * p + 1) * 128],
                            qtbh[h][:], start=True, stop=True,
                        )
                        nc.tensor.matmul(
                            ps_s[:, 512:1024],
                            ktfg[g][:, (2 * p + 1) * 128:(2 * p + 2) * 128],
                            qtbh[h][:], start=True, stop=True,
                        )
                        a = blk.tile([128, 1024], BF16, tag="blk")
                        nc.scalar.activation(a[:], ps_s[:], AF.Exp)
                        pairs.append(a)
                        nc.tensor.matmul(
                            ps_y[:], vfr[2 * p][:, gs0:gs1], a[:, 0:512],
                            start=(p == 0), stop=False, skip_group_check=True,
                        )
                        nc.tensor.matmul(
                            ps_y[:], vfr[2 * p + 1][:, gs0:gs1], a[:, 512:1024],
                            start=False, stop=(p == 7), skip_group_check=True,
                        )
                    return pairs, ps_y

                def attn_tree(pairs):
                    # in-place pairwise tree: pairs[0] <- sum of all 8
                    for step in (1, 2, 4):
                        for base in range(0, 8, 2 * step):
                            nc.vector.tensor_tensor(
                                pairs[base][:], pairs[base][:],
                                pairs[base + step][:], op=ALU.add,
                            )

                def attn_norm(h, pairs, ps_y):
                    ps_S = pss.tile([1, 512], F32, tag="pss")
                    nc.tensor.matmul(
                        ps_S[:], ones_colb[:], pairs[0][:, 0:512],
                        start=True, stop=False,
                    )
                    nc.tensor.matmul(
                        ps_S[:], ones_colb[:], pairs[0][:, 512:1024],
                        start=False, stop=True, skip_group_check=True,
                    )
                    rS = s1.tile([1, 512], F32R, tag="s1")
                    with nc.allow_low_precision("fp32r rounding for bcast matmul"):
                        nc.vector.reciprocal(rS[:], ps_S[:])
                    ps_r = pss.tile([128, 512], F32, tag="pss")
                    nc.tensor.matmul(
                        ps_r[:], ones_row[0:1, 0:128], rS[:],
                        start=True, stop=True,
                    )
                    rSb = blkf.tile([128, 512], F32, tag="blkf")
                    nc.vector.tensor_copy(rSb[:], ps_r[:])
                    nc.vector.tensor_tensor(
                        yt[:, h, :], ps_y[:], rSb[:], op=ALU.mult
                    )

                def ln_sums(ct, ps_mu, ps_sq):
                    ysq = blk.tile([128, 512], BF16, tag="blk", name=f"ysq{ct}")
                    nc.vector.tensor_tensor(
                        ysq[:], yt[:, ct, :], yt[:, ct, :], op=ALU.mult
                    )
                    nc.tensor.matmul(
                        ps_mu[:], ones_colb[:], yt[:, ct, :],
                        start=(ct == 0), stop=(ct == 15), skip_group_check=True,
                    )
                    nc.tensor.matmul(
                        ps_sq[:], ones_colb[:], ysq[:],
                        start=(ct == 0), stop=(ct == 15), skip_group_check=True,
                    )

                def attn_tail(h, pairs, ps_y):
                    attn_tree(pairs)
                    attn_norm(h, pairs, ps_y)

                pending = None
                for h in range(H):
                    if h + 2 < H:
                        qtbh.append(qproj(h + 2))
                    cur = attn_head(h)
                    if pending is not None:
                        attn_tail(h - 1, *pending)
                    pending = cur

                # ---- last head tail + LayerNorm sums, pipelined against the
                # first out-proj group's independent matmuls ----
                attn_tree(pending[0])
                # z(jb0,m0) i=0..14 needs only yt[0..14] + wob[0]: covers the
                # tree on PE
                ps_o00 = ps.tile([128, 512], F32, tag="ps")
                for i in range(15):
                    nc.tensor.matmul(
                        ps_o00[:], yt[:, i, 0:128], wob[0][:, i, :],
                        start=(i == 0), stop=False, skip_group_check=True,
                    )
                attn_norm(H - 1, *pending)
                ps_lns = ps.tile([1, 1024], F32, tag="ps")
                ps_mu = ps_lns[:, 0:512]
                ps_sq = ps_lns[:, 512:1024]
                # mu sums first: PE-independent work that covers the last
                # head's DVE chain; the DVE-paced squared sums follow
                for ct in range(15):
                    nc.tensor.matmul(
                        ps_mu[:], ones_colb[:], yt[:, ct, :],
                        start=(ct == 0), stop=False, skip_group_check=True,
                    )
                for ct in range(16):
                    ysq = blk.tile([128, 512], BF16, tag="blk", name=f"ysq{ct}")
                    nc.vector.tensor_tensor(
                        ysq[:], yt[:, ct, :], yt[:, ct, :], op=ALU.mult
                    )
                    nc.tensor.matmul(
                        ps_sq[:], ones_colb[:], ysq[:],
                        start=(ct == 0), stop=(ct == 15), skip_group_check=True,
                    )
                nc.tensor.matmul(
                    ps_mu[:], ones_colb[:], yt[:, 15, :],
                    start=False, stop=True, skip_group_check=True,
                )
                # one DVE op drains both PSUM chains -> frees the ps slot for
                # the next out-proj group ~1.3us earlier
                mum2 = rsc.tile([1, 1024], F32, tag="mum2")
                nc.vector.tensor_scalar_mul(mum2[:], ps_lns[:], 1.0 / C)
                mu = mum2[:, 0:512]
                m2 = mum2[:, 512:1024]
                negmu = rsc.tile([1, 512], BF16, tag="negmu")
                nc.vector.tensor_scalar_mul(negmu[:], mu, -1.0)
                var = s1.tile([1, 512], F32, tag="s1")
                nc.vector.tensor_tensor(var[:], mu, mu, op=ALU.mult)
                nc.vector.tensor_tensor(var[:], m2, var[:], op=ALU.subtract)
                nc.vector.tensor_scalar_add(var[:], var[:], EPS)
                sd = s1.tile([1, 512], F32, tag="s1")
                nc.scalar.activation(sd[:], var[:], AF.Sqrt)
                rstd = s1.tile([1, 512], F32R, tag="s1")
                with nc.allow_low_precision("fp32r rounding for transpose mm"):
                    nc.vector.reciprocal(rstd[:], sd[:])
                # rstd [1,512] -> [128,4] (row r = m*128+p at [p, m]) on PE:
                # 4 small matmuls (row-slice^T x ones) — no DRAM round-trip
                # on the critical path
                rstd_c = rsc.tile([128, 4], F32)
                for m in range(4):
                    ps_rc = pss.tile([128, 4], F32, tag="pss", name=f"psrc{m}")
                    nc.tensor.matmul(
                        ps_rc[:], rstd[0:1, m * 128:(m + 1) * 128],
                        ones_row[0:1, 0:4], start=True, stop=True,
                    )
                    nc.vector.tensor_copy(rstd_c[:, m:m + 1], ps_rc[:, 0:1])

                # ---- output projection with folded LayerNorm ----
                for jb in range(4):
                    if wob[jb] is None:
                        wob[jb] = xt.tile([128, 16, 512], BF16, tag="xt",
                                          name=f"wob{jb}")
                        nc.sync.dma_start(out=wob[jb][:], in_=wo_d[jb])
                    for m in range(4):
                        if jb == 0 and m == 0:
                            # group opened pre-stats (covered the last head's
                            # tail); finish with i=15
                            ps_o = ps_o00
                            nc.tensor.matmul(
                                ps_o[:], yt[:, 15, 0:128], wob[0][:, 15, :],
                                start=False, stop=False, skip_group_check=True,
                            )
                        else:
                            ps_o = ps.tile([128, 512], F32, tag="ps")
                            for i in range(16):
                                nc.tensor.matmul(
                                    ps_o[:], yt[:, i, m * 128:(m + 1) * 128],
                                    wob[jb][:, i, :], start=(i == 0), stop=False,
                                    skip_group_check=True,
                                )
                        # z - mu (x) s_w : rank-1 correction rides the group
                        nc.tensor.matmul(
                            ps_o[:], negmu[0:1, m * 128:(m + 1) * 128],
                            sw_sb[0:1, jb * 512:(jb + 1) * 512],
                            start=False, stop=True,
                        )
                        osb = blkf.tile([128, 512], F32, tag="blkf")
                        nc.vector.tensor_scalar(
                            osb[:], ps_o[:], rstd_c[:, m:m + 1], None,
                            op0=ALU.mult,
                        )
                        nc.vector.tensor_tensor(
                            osb[:], osb[:], lnbo_b[jb][:], op=ALU.add
                        )
                        nc.scalar.dma_start(
                            out=out_d[m * 128:(m + 1) * 128, jb * 512:(jb + 1) * 512],
                            in_=osb[:],
                        )

    nc.compile()
    return nc


_NC_CACHE = None


def _get_nc():
    global _NC_CACHE
    if _NC_CACHE is None:
        _NC_CACHE = build_kernel()
    return _NC_CACHE


def _prep_shared(Wq, bq, Wk, bk, Wv, bv, ln_w, ln_b, Wo, bo):
    import ml_dtypes

    bf = ml_dtypes.bfloat16
    s = np.float32(SCALE)
    WqT = np.ascontiguousarray(Wq.T) * s  # [c, ch], scale folded into q
    wq = np.ascontiguousarray(
        WqT.reshape(16, 128, 16, 128).transpose(2, 1, 0, 3)
    ).astype(bf)
    WkT = np.ascontiguousarray(Wk.T)  # [2048, 512]
    wk = np.ascontiguousarray(
        WkT.reshape(16, 128, 4, 128).transpose(1, 2, 0, 3)
    ).astype(bf)
    WvT = np.ascontiguousarray(Wv.T)  # [2048, 512]
    wv = np.ascontiguousarray(
        WvT.reshape(16, 128, KVC).transpose(1, 0, 2)
    ).astype(bf)
    WoT = np.ascontiguousarray(Wo.T)  # [2048, 2048]
    WoTs = ln_w[:, None].astype(np.float32) * WoT  # ln_w folded in
    wo = np.ascontiguousarray(
        WoTs.reshape(16, 128, 4, 512).transpose(2, 1, 0, 3)
    ).astype(bf)
    s_w = ln_w @ WoT  # [2048]
    lnbo = ln_b @ WoT + bo  # [2048]

    cf = np.zeros((128, 20), np.float32)
    cf[:, 0:16] = (bq * s).reshape(16, 128).T
    cf[:, 16:20] = bk.reshape(4, 128).T
    crb = np.zeros((1, 5120), np.float32)
    crb[0, 0:KVC] = bv
    crb[0, KVC:KVC + C] = s_w
    crb[0, KVC + C:KVC + 2 * C] = lnbo
    crb[0, KVC + 2 * C:KVC + 2 * C + 512] = 1.0
    return {
        "wq": wq,
        "wk": wk,
        "wv": wv,
        "wo": wo,
        "cf": cf,
        "crb": crb.astype(bf),
        "onesb": np.ones((128, 1), bf),
        "onesr": np.ones((1, 512), np.float32),
    }


def _xt_tiled(x):
    # x [R_, C] -> x^T tiled [128, 16, R_] bf16
    import ml_dtypes

    xT = np.ascontiguousarray(x.T)  # [C, R_]
    return np.ascontiguousarray(
        xT.reshape(16, 128, x.shape[0]).transpose(1, 0, 2)
    ).astype(ml_dtypes.bfloat16)


def kernel(
    query, key, value, Wq, bq, Wk, bk, Wv, bv, ln_w, ln_b, Wo, bo
):
    query = np.asarray(query, np.float32)
    key = np.asarray(key, np.float32)
    value = np.asarray(value, np.float32)

    nc = _get_nc()
    shared = _prep_shared(
        np.asarray(Wq, np.float32), np.asarray(bq, np.float32),
        np.asarray(Wk, np.float32), np.asarray(bk, np.float32),
        np.asarray(Wv, np.float32), np.asarray(bv, np.float32),
        np.asarray(ln_w, np.float32), np.asarray(ln_b, np.float32),
        np.asarray(Wo, np.float32), np.asarray(bo, np.float32),
    )

    xkT = [_xt_tiled(key[b]) for b in range(B)]
    xvT = [_xt_tiled(value[b]) for b in range(B)]

    in_maps = []
    for c in range(N_CORES):
        b = c // 4
        r0 = (c % 4) * R
        m = dict(shared)
        m["xq"] = _xt_tiled(query[b, r0:r0 + R, :])
        m["xk"] = xkT[b]
        m["xv"] = xvT[b]
        in_maps.append(m)

    res = run_bass_kernel_spmd(nc, in_maps, core_ids=list(range(N_CORES)))
    global LAST_RESULT
    LAST_RESULT = res

    out = np.empty((B, T, C), np.float32)
    for c in range(N_CORES):
        b = c // 4
        r0 = (c % 4) * R
        out[b, r0:r0 + R, :] = res.results[c]["out"]
    return out


# revision 52
# speedup vs baseline: 1.0344x; 1.0344x over previous
"""BitMGQA forward on 8 trn2 NeuronCores — collective-free.

Core c owns batch b=c//4 and query rows (c%4)*512:(c%4+1)*512. Each core
recomputes the K/V projections for its batch's FULL 2048-key sequence
locally (instead of all-gathering K/V across the batch group), so there are
no cross-core dependencies at all: each core's on-device instruction span
is just its own work, immune to multi-core launch skew.

All matmul operands are bf16 (fp32 PSUM accumulation). LayerNorm is folded
into the output projection: with z = y @ (ln_w*WoT), s_w = ln_w @ WoT,
lnbo = ln_b @ WoT + bo, the output is rstd*(z - mu*s_w) + lnbo, so the
-mu*s_w term rides the z PSUM accumulation and the per-row rstd scale plus
lnbo add are two DVE ops per output tile. Outputs are disjoint row
slices -> host concat.
"""

import numpy as np

import concourse.bacc as bacc
import concourse.mybir as mybir
import concourse.tile as tile
from concourse.bass_utils import run_bass_kernel_spmd

B, T, C = 2, 2048, 2048
H, KV = 16, 4
HD = C // H  # 128
KVC = HD * KV  # 512
EPS = 1e-5
R = 512  # query rows per core
N_CORES = 8
SCALE = 1.0 / np.sqrt(HD)

F32 = mybir.dt.float32
F32R = mybir.dt.float32r
BF16 = mybir.dt.bfloat16
AF = mybir.ActivationFunctionType
ALU = mybir.AluOpType


def build_kernel(loop_n=1):
    nc = bacc.Bacc(
        "TRN2", target_bir_lowering=False, debug=False, num_devices=N_CORES
    )

    # Per-core inputs (host pre-transposed/tiled, see kernel() below)
    xq_d = nc.dram_tensor("xq", [128, 16, R], BF16, kind="ExternalInput").ap()
    xk_d = nc.dram_tensor("xk", [128, 16, T], BF16, kind="ExternalInput").ap()
    xv_d = nc.dram_tensor("xv", [128, 16, T], BF16, kind="ExternalInput").ap()
    # wq[j] = [128, 16, 128] (c-within-tile, c-tile, ch) for ch-tile j
    wq_d = nc.dram_tensor("wq", [16, 128, 16, 128], BF16, kind="ExternalInput").ap()
    wk_d = nc.dram_tensor("wk", [128, 4, 16, 128], BF16, kind="ExternalInput").ap()
    wv_d = nc.dram_tensor("wv", [128, 16, KVC], BF16, kind="ExternalInput").ap()
    wo_d = nc.dram_tensor("wo", [4, 128, 16, 512], BF16, kind="ExternalInput").ap()
    # packed consts: cf = [bq(16) | bk(4)] f32; crb = [bv(512) | sw(2048) |
    # lnbo(2048) | ones(512)] bf16
    cf_d = nc.dram_tensor("cf", [128, 20], F32, kind="ExternalInput").ap()
    crb_d = nc.dram_tensor("crb", [1, 5120], BF16, kind="ExternalInput").ap()
    onesb_d = nc.dram_tensor("onesb", [128, 1], BF16, kind="ExternalInput").ap()
    onesr_d = nc.dram_tensor("onesr", [1, 512], F32R, kind="ExternalInput").ap()

    out_d = nc.dram_tensor("out", [R, C], F32, kind="ExternalOutput").ap()

    from contextlib import ExitStack

    with tile.TileContext(nc) as tc:
        with ExitStack() as stack:
            ep = stack.enter_context
            consts = ep(tc.tile_pool(name="consts", bufs=1))
            dram = ep(tc.tile_pool(name="dram", bufs=1, space="DRAM"))
            xt = ep(tc.tile_pool(name="xt", bufs=2))            # [128,16,512] bf16
            wkb_pool = ep(tc.tile_pool(name="wkb", bufs=1))     # [128,4,16,128] bf16
            wqb_pool = ep(tc.tile_pool(name="wqb", bufs=2))     # [128,16,128] bf16
            wvb_pool = ep(tc.tile_pool(name="wvb", bufs=1))     # [128,16,512] bf16
            ktf_pool = ep(tc.tile_pool(name="ktf", bufs=4))     # [128,2048] bf16
            vfb_pool = ep(tc.tile_pool(name="vfb", bufs=16))    # [128,512] bf16
            qtb_pool = ep(tc.tile_pool(name="qtb", bufs=16))    # [128,512] bf16
            blk = ep(tc.tile_pool(name="blk", bufs=12))         # [128,1024] bf16
            blkf = ep(tc.tile_pool(name="blkf", bufs=3))        # [128,512] f32
            lnbo_pool = ep(tc.tile_pool(name="lnbo", bufs=4))   # [128,512] bf16
            ytp = ep(tc.tile_pool(name="ytp", bufs=1))          # [128,16,512] bf16
            s1 = ep(tc.tile_pool(name="s1", bufs=3))            # [1,512] f32
            rsc = ep(tc.tile_pool(name="rsc", bufs=1))          # [128,4] f32
            # PSUM: ps slots are [128,1024] (2 banks); 2x2 + 2 + 2 = 8 banks
            ps = ep(tc.tile_pool(name="ps", bufs=2, space="PSUM"))    # [128,1024]
            psy = ep(tc.tile_pool(name="psy", bufs=2, space="PSUM"))  # [128,512]
            pss = ep(tc.tile_pool(name="pss", bufs=2, space="PSUM"))  # [1,512]
            for _it in range(loop_n):
                # ---- bulk input streams first; wk on the ACT ring so it
                # transfers in parallel with xkq0 on the SP ring. Both are
                # split so the first K-proj group's operands land first.
                wk_sb = wkb_pool.tile([128, 4, 16, 128], BF16, tag="wkb")
                nc.scalar.dma_start(out=wk_sb[:, 0:1], in_=wk_d[:, 0:1])
                nc.scalar.dma_start(out=wk_sb[:, 1:4], in_=wk_d[:, 1:4])
                wkb = [wk_sb[:, g] for g in range(4)]
                xkq0 = xt.tile([128, 16, 512], BF16, tag="xt", name="xkq0")
                nc.sync.dma_start(out=xkq0[:, 0:8], in_=xk_d[:, 0:8, 0:512])
                nc.sync.dma_start(out=xkq0[:, 8:16], in_=xk_d[:, 8:16, 0:512])

                # ---- consts on the ACT HWDGE ring (parallel to SP's) ----
                cf = consts.tile([128, 20], F32)
                nc.scalar.dma_start(out=cf[:], in_=cf_d[:])
                bq_sb = cf[:, 0:16]
                bk_sb = cf[:, 16:20]
                crb = consts.tile([1, 5120], BF16)
                nc.scalar.dma_start(out=crb[:], in_=crb_d[:])
                bv_sb = crb[:, 0:KVC]
                sw_sb = crb[:, KVC:KVC + C]
                lnbo_sb = crb[:, KVC + C:KVC + 2 * C]
                ones_rowb = crb[:, KVC + 2 * C:KVC + 2 * C + 512]
                ones_colb = consts.tile([128, 1], BF16)
                nc.scalar.dma_start(out=ones_colb[:], in_=onesb_d[:])
                ones_row = consts.tile([1, 512], F32R)
                nc.scalar.dma_start(out=ones_row[:], in_=onesr_d[:])

                # ---- K projection (k^T layout), full T, streamed t-quarters ----
                ktfg = []
                for g in range(4):
                    ktfg.append(ktf_pool.tile([128, T], BF16, tag="ktfg",
                                              name=f"ktfg{g}"))
                for tq in range(4):
                    if tq == 0:
                        xkq = xkq0
                    else:
                        xkq = xt.tile([128, 16, 512], BF16, tag="xt",
                                      name=f"xkq{tq}")
                        nc.sync.dma_start(
                            out=xkq[:], in_=xk_d[:, :, tq * 512:(tq + 1) * 512]
                        )
                    for g in range(4):
                        ps_k = ps.tile([128, 512], F32, tag="ps")
                        for i in range(16):
                            nc.tensor.matmul(
                                ps_k[:], wkb[g][:, i, :], xkq[:, i, :],
                                start=(i == 0), stop=(i == 15),
                                skip_group_check=True,
                            )
                        nc.scalar.activation(
                            ktfg[g][:, tq * 512:(tq + 1) * 512], ps_k[:],
                            AF.Identity, bias=bk_sb[:, g:g + 1],
                        )

                # ---- V projection (natural layout), full T, streamed ----
                wv_sb = wvb_pool.tile([128, 16, KVC], BF16)
                nc.sync.dma_start(out=wv_sb[:], in_=wv_d[:])
                vfr = []
                for tq in range(4):
                    xvq = xt.tile([128, 16, 512], BF16, tag="xt", name=f"xvq{tq}")
                    nc.sync.dma_start(out=xvq[:], in_=xv_d[:, :, tq * 512:(tq + 1) * 512])
                    for tt in range(4):
                        ps_v = ps.tile([128, 512], F32, tag="ps")
                        nc.tensor.matmul(
                            ps_v[:], ones_rowb[0:1, 0:128], bv_sb[0:1, :],
                            start=True, stop=False,
                        )
                        for i in range(16):
                            nc.tensor.matmul(
                                ps_v[:], xvq[:, i, tt * 128:(tt + 1) * 128],
                                wv_sb[:, i, :], start=False, stop=(i == 15),
                                skip_group_check=True,
                            )
                        vt = vfb_pool.tile([128, KVC], BF16, tag="vfr",
                                           name=f"vfr{tq*4+tt}")
                        nc.scalar.activation(vt[:], ps_v[:], AF.Copy)
                        vfr.append(vt)

                # ---- Q projection is interleaved into the attention loop ----
                xq = xt.tile([128, 16, R], BF16, tag="xt", name="xq")
                nc.sync.dma_start(out=xq[:], in_=xq_d[:])

                def qproj(j):
                    wqb = wqb_pool.tile([128, 16, 128], BF16, tag="wqb")
                    nc.sync.dma_start(out=wqb[:], in_=wq_d[j])
                    ps_q = ps.tile([128, 512], F32, tag="ps")
                    for i in range(16):
                        nc.tensor.matmul(
                            ps_q[:], wqb[:, i, :], xq[:, i, :],
                            start=(i == 0), stop=(i == 15), skip_group_check=True,
                        )
                    qh = qtb_pool.tile([128, R], BF16, tag="qtbh", name=f"qtbh{j}")
                    nc.scalar.activation(
                        qh[:], ps_q[:], AF.Identity, bias=bq_sb[:, j:j + 1]
                    )
                    return qh

                # first two heads' q up front so attention can start
                qtbh = [qproj(0), qproj(1)]

                # ---- prefetch first Wo blocks; lnbo broadcast tiles ----
                wob = [None] * 4
                for jb in range(2):
                    wob[jb] = xt.tile([128, 16, 512], BF16, tag="xt",
                                      name=f"wob{jb}")
                    nc.sync.dma_start(out=wob[jb][:], in_=wo_d[jb])
                lnbo_b = []
                for jb in range(4):
                    ps_l = ps.tile([128, 512], F32, tag="ps")
                    nc.tensor.matmul(
                        ps_l[:], ones_rowb[0:1, 0:128],
                        lnbo_sb[0:1, jb * 512:(jb + 1) * 512],
                        start=True, stop=True,
                    )
                    lb = lnbo_pool.tile([128, 512], BF16, tag="lnbo",
                                        name=f"lnbo{jb}")
                    nc.scalar.activation(lb[:], ps_l[:], AF.Copy)
                    lnbo_b.append(lb)

                # ---- attention: paired score tiles, DVE softmax-sum tree.
                # Software-pipelined: head h's tail (tree -> S -> 1/S ->
                # bcast -> normalize) is emitted AFTER head h+1's score/AV
                # block, so the PE queue always holds independent matmuls
                # while the DVE tree catches up (no head-of-line stalls).
                yt = ytp.tile([128, 16, R], BF16)

                def attn_head(h):
                    g = h // 4
                    gs0, gs1 = g * 128, (g + 1) * 128
                    ps_y = psy.tile([128, 512], F32, tag="psy")
                    pairs = []
                    for p in range(8):
                        ps_s = ps.tile([128, 1024], F32, tag="ps")
                        nc.tensor.matmul(
                            ps_s[:, 0:512],
                            ktfg[g][:, (2 * p) * 128:(2 * p + 1) * 128],
                            qtbh[h][:], start=True, stop=True,
                        )
                        nc.tensor.matmul(
                            ps_s[:, 512:1024],
                            ktfg[g][:, (2 * p + 1) * 128:(2 * p + 2) * 128],
                            qtbh[h][:], start=True, stop=True,
                        )
                        a = blk.tile([128, 1024], BF16, tag="blk")
                        nc.scalar.activation(a[:], ps_s[:], AF.Exp)
                        pairs.append(a)
                        nc.tensor.matmul(
                            ps_y[:], vfr[2 * p][:, gs0:gs1], a[:, 0:512],
                            start=(p == 0), stop=False, skip_group_check=True,
                        )
                        nc.tensor.matmul(
                            ps_y[:], vfr[2 * p + 1][:, gs0:gs1], a[:, 512:1024],
                            start=False, stop=(p == 7), skip_group_check=True,
                        )
                    return pairs, ps_y

                def attn_tree(pairs):
                    # in-place pairwise tree: pairs[0] <- sum of all 8
                    for step in (1, 2, 4):
                        for base in range(0, 8, 2 * step):
                            nc.vector.tensor_tensor(
                                pairs[base][:], pairs[base][:],
                                pairs[base + step][:], op=ALU.add,
                            )

                def attn_norm(h, pairs, ps_y):
                    ps_S = pss.tile([1, 512], F32, tag="pss")
                    nc.tensor.matmul(
                        ps_S[:], ones_colb[:], pairs[0][:, 0:512],
                        start=True, stop=False,
                    )
                    nc.tensor.matmul(
                        ps_S[:], ones_colb[:], pairs[0][:, 512:1024],
                        start=False, stop=True, skip_group_check=True,
                    )
                    rS = s1.tile([1, 512], F32R, tag="s1")
                    with nc.allow_low_precision("fp32r rounding for bcast matmul"):
                        nc.vector.reciprocal(rS[:], ps_S[:])
                    ps_r = pss.tile([128, 512], F32, tag="pss")
                    nc.tensor.matmul(
                        ps_r[:], ones_row[0:1, 0:128], rS[:],
                        start=True, stop=True,
                    )
                    rSb = blkf.tile([128, 512], F32, tag="blkf")
                    nc.vector.tensor_copy(rSb[:], ps_r[:])
                    nc.vector.tensor_tensor(
                        yt[:, h, :], ps_y[:], rSb[:], op=ALU.mult
                    )

                def ln_sums(ct, ps_mu, ps_sq):
                    ysq = blk.tile([128, 512], BF16, tag="blk", name=f"ysq{ct}")
                    nc.vector.tensor_tensor(
                        ysq[:], yt[:, ct, :], yt[:, ct, :], op=ALU.mult
                    )
                    nc.tensor.matmul(
                        ps_mu[:], ones_colb[:], yt[:, ct, :],
                        start=(ct == 0), stop=(ct == 15), skip_group_check=True,
                    )
                    nc.tensor.matmul(
                        ps_sq[:], ones_colb[:], ysq[:],
                        start=(ct == 0), stop=(ct == 15), skip_group_check=True,
                    )

                def attn_tail(h, pairs, ps_y):
                    attn_tree(pairs)
                    attn_norm(h, pairs, ps_y)

                pending = None
                for h in range(H):
                    if h + 2 < H:
                        qtbh.append(qproj(h + 2))
                    cur = attn_head(h)
                    if pending is not None:
                        attn_tail(h - 1, *pending)
                    pending = cur

                # ---- last head tail + LayerNorm sums, pipelined against the
                # first out-proj group's independent matmuls ----
                attn_tree(pending[0])
                # z(jb0,m0) i=0..14 needs only yt[0..14] + wob[0]: covers the
                # tree on PE
                ps_o00 = ps.tile([128, 512], F32, tag="ps")
                for i in range(15):
                    nc.tensor.matmul(
                        ps_o00[:], yt[:, i, 0:128], wob[0][:, i, :],
                        start=(i == 0), stop=False, skip_group_check=True,
                    )
                attn_norm(H - 1, *pending)
                ps_lns = ps.tile([1, 1024], F32, tag="ps")
                ps_mu = ps_lns[:, 0:512]
                ps_sq = ps_lns[:, 512:1024]
                # mu sums first: PE-independent work that covers the last
                # head's DVE chain; the DVE-paced squared sums follow
                for ct in range(15):
                    nc.tensor.matmul(
                        ps_mu[:], ones_colb[:], yt[:, ct, :],
                        start=(ct == 0), stop=False, skip_group_check=True,
                    )
                for ct in range(16):
                    ysq = blk.tile([128, 512], BF16, tag="blk", name=f"ysq{ct}")
                    nc.vector.tensor_tensor(
                        ysq[:], yt[:, ct, :], yt[:, ct, :], op=ALU.mult
                    )
                    nc.tensor.matmul(
                        ps_sq[:], ones_colb[:], ysq[:],
                        start=(ct == 0), stop=(ct == 15), skip_group_check=True,
                    )
                nc.tensor.matmul(
                    ps_mu[:], ones_colb[:], yt[:, 15, :],
                    start=False, stop=True, skip_group_check=True,
                )
                # one DVE op drains both PSUM chains -> frees the ps slot for
                # the next out-proj group ~1.3us earlier
                mum2 = rsc.tile([1, 1024], F32, tag="mum2")
                nc.vector.tensor_scalar_mul(mum2[:], ps_lns[:], 1.0 / C)
                mu = mum2[:, 0:512]
                m2 = mum2[:, 512:1024]
                negmu = rsc.tile([1, 512], BF16, tag="negmu")
                nc.vector.tensor_scalar_mul(negmu[:], mu, -1.0)
                var = s1.tile([1, 512], F32, tag="s1")
                nc.vector.tensor_tensor(var[:], mu, mu, op=ALU.mult)
                nc.vector.tensor_tensor(var[:], m2, var[:], op=ALU.subtract)
                nc.vector.tensor_scalar_add(var[:], var[:], EPS)
                sd = s1.tile([1, 512], F32, tag="s1")
                nc.scalar.activation(sd[:], var[:], AF.Sqrt)
                rstd = s1.tile([1, 512], F32R, tag="s1")
                with nc.allow_low_precision("fp32r rounding for transpose mm"):
                    nc.vector.reciprocal(rstd[:], sd[:])
                # rstd [1,512] -> [128,4] (row r = m*128+p at [p, m]) on PE:
                # 4 small matmuls (row-slice^T x ones) — no DRAM round-trip
                # on the critical path
                rstd_c = rsc.tile([128, 4], F32)
                for m in range(4):
                    ps_rc = pss.tile([128, 4], F32, tag="pss", name=f"psrc{m}")
                    nc.tensor.matmul(
                        ps_rc[:], rstd[0:1, m * 128:(m + 1) * 128],
                        ones_row[0:1, 0:4], start=True, stop=True,
                    )
                    nc.vector.tensor_copy(rstd_c[:, m:m + 1], ps_rc[:, 0:1])

                # ---- output projection with folded LayerNorm ----
                for jb in range(4):
                    if wob[jb] is None:
                        wob[jb] = xt.tile([128, 16, 512], BF16, tag="xt",
                                          name=f"wob{jb}")
                        nc.sync.dma_start(out=wob[jb][:], in_=wo_d[jb])
                    for m in range(4):
                        if jb == 0 and m == 0:
                            # group opened pre-stats (covered the last head's
                            # tail); finish with i=15
                            ps_o = ps_o00
                            nc.tensor.matmul(
                                ps_o[:], yt[:, 15, 0:128], wob[0][:, 15, :],
                                start=False, stop=False, skip_group_check=True,
                            )
                        else:
                            ps_o = ps.tile([128, 512], F32, tag="ps")
                            for i in range(16):
                                nc.tensor.matmul(
                                    ps_o[:], yt[:, i, m * 128:(m + 1) * 128],
                                    wob[jb][:, i, :], start=(i == 0), stop=False,
                                    skip_group_check=True,
                                )
                        # z - mu (x) s_w : rank-1 correction rides the group
                        nc.tensor.matmul(
                            ps_o[:], negmu[0:1, m * 128:(m + 1) * 128],
                            sw_sb[0:1, jb * 512:(jb + 1) * 512],
                            start=False, stop=True,
                        )
                        osb = blkf.tile([128, 512], F32, tag="blkf")
                        nc.vector.tensor_scalar(
                            osb[:], ps_o[:], rstd_c[:, m:m + 1], None,
                            op0=ALU.mult,
                        )
                        nc.vector.tensor_tensor(
                            osb[:], osb[:], lnbo_b[jb][:], op=ALU.add
                        )
                        nc.scalar.dma_start(
                            out=out_d[m * 128:(m + 1) * 128, jb * 512:(jb + 1) * 512],
                            in_=osb[:],
                        )

    nc.compile()
    return nc


_NC_CACHE = None


def _get_nc():
    global _NC_CACHE
    if _NC_CACHE is None:
        _NC_CACHE = build_kernel()
    return _NC_CACHE


def _prep_shared(Wq, bq, Wk, bk, Wv, bv, ln_w, ln_b, Wo, bo):
    import ml_dtypes

    bf = ml_dtypes.bfloat16
    s = np.float32(SCALE)
    WqT = np.ascontiguousarray(Wq.T) * s  # [c, ch], scale folded into q
    wq = np.ascontiguousarray(
        WqT.reshape(16, 128, 16, 128).transpose(2, 1, 0, 3)
    ).astype(bf)
    WkT = np.ascontiguousarray(Wk.T)  # [2048, 512]
    wk = np.ascontiguousarray(
        WkT.reshape(16, 128, 4, 128).transpose(1, 2, 0, 3)
    ).astype(bf)
    WvT = np.ascontiguousarray(Wv.T)  # [2048, 512]
    wv = np.ascontiguousarray(
        WvT.reshape(16, 128, KVC).transpose(1, 0, 2)
    ).astype(bf)
    WoT = np.ascontiguousarray(Wo.T)  # [2048, 2048]
    WoTs = ln_w[:, None].astype(np.float32) * WoT  # ln_w folded in
    wo = np.ascontiguousarray(
        WoTs.reshape(16, 128, 4, 512).transpose(2, 1, 0, 3)
    ).astype(bf)
    s_w = ln_w @ WoT  # [2048]
    lnbo = ln_b @ WoT + bo  # [2048]

    cf = np.zeros((128, 20), np.float32)
    cf[:, 0:16] = (bq * s).reshape(16, 128).T
    cf[:, 16:20] = bk.reshape(4, 128).T
    crb = np.zeros((1, 5120), np.float32)
    crb[0, 0:KVC] = bv
    crb[0, KVC:KVC + C] = s_w
    crb[0, KVC + C:KVC + 2 * C] = lnbo
    crb[0, KVC + 2 * C:KVC + 2 * C + 512] = 1.0
    return {
        "wq": wq,
        "wk": wk,
        "wv": wv,
        "wo": wo,
        "cf": cf,
        "crb": crb.astype(bf),
        "onesb": np.ones((128, 1), bf),
        "onesr": np.ones((1, 512), np.float32),
    }


def _xt_tiled(x):
    # x [R_, C] -> x^T tiled [128, 16, R_] bf16
    import ml_dtypes

    xT = np.ascontiguousarray(x.T)  # [C, R_]
    return np.ascontiguousarray(
        xT.reshape(16, 128, x.shape[0]).transpose(1, 0, 2)
    ).astype(ml_dtypes.bfloat16)


def kernel(
    query, key, value, Wq, bq, Wk, bk, Wv, bv, ln_w, ln_b, Wo, bo
):
    query = np.asarray(query, np.float32)
    key = np.asarray(key, np.float32)
    value = np.asarray(value, np.float32)

    nc = _get_nc()
    shared = _prep_shared(
        np.asarray(Wq, np.float32), np.asarray(bq, np.float32),
        np.asarray(Wk, np.float32), np.asarray(bk, np.float32),
        np.asarray(Wv, np.float32), np.asarray(bv, np.float32),
        np.asarray(ln_w, np.float32), np.asarray(ln_b, np.float32),
        np.asarray(Wo, np.float32), np.asarray(bo, np.float32),
    )

    xkT = [_xt_tiled(key[b]) for b in range(B)]
    xvT = [_xt_tiled(value[b]) for b in range(B)]

    in_maps = []
    for c in range(N_CORES):
        b = c // 4
        r0 = (c % 4) * R
        m = dict(shared)
        m["xq"] = _xt_tiled(query[b, r0:r0 + R, :])
        m["xk"] = xkT[b]
        m["xv"] = xvT[b]
        in_maps.append(m)

    res = run_bass_kernel_spmd(nc, in_maps, core_ids=list(range(N_CORES)))
    global LAST_RESULT
    LAST_RESULT = res

    out = np.empty((B, T, C), np.float32)
    for c in range(N_CORES):
        b = c // 4
        r0 = (c % 4) * R
        out[b, r0:r0 + R, :] = res.results[c]["out"]
    return out
